# revision 1
# baseline (speedup 1.0000x reference)
"""AA_GAT on 8 trn2 cores (self-contained).

Strategy: edges sharded by src-range (6272 nodes/core, 49 windows of 128
nodes). Host does layout only: sort/bucket edges per (core, window,
tgt-half), pad to 128-chunks, wrap arrays for the device. Per-core tables
(h per head, attention alphas) are stored rotated by -core_base so every
core addresses its own nodes at rows [0, 6272) with static gather bases.

Device launch 1: node MLP, per-head h + alpha tables, edge MLP -> edge
scores, per-window: dma_gather of alpha/h rows, attention weights, one-hot
S matmul segment-sum (num+den per window PSUM), elu -> xh, h_out = xh@out_W
+ out-layer alphas.  Host: concat h_out slices.  Launch 2: out-layer edge
pass + log_softmax.
"""

import os
import numpy as np

import concourse.bass as bass
import concourse.mybir as mybir
import concourse.tile as tile
from concourse import bacc
from concourse.bass_utils import run_bass_kernel_spmd
from concourse.masks import make_identity

F32 = mybir.dt.float32
BF16 = mybir.dt.bfloat16
I16 = mybir.dt.int16
I32 = mybir.dt.int32
AF = mybir.ActivationFunctionType
OP = mybir.AluOpType
AX = mybir.AxisListType

N = 50000
E = 1_000_000
NODE_DIM = 16
EMB = 64
OUT = 64
HEADS = 8
EA_DIM = 8
SLOPE = 0.01
LN_EPS = 1e-5

NCORES = 8
NPC = 6272            # nodes per core (49*128)
NWIN = 49
NPAD = 50048          # 391*128
NT = NPAD // 128      # 391 node tiles
TSPLIT = 32768
HIB = NPAD - TSPLIT   # 17280 (hi gather base row)


def _wrap_slots(a):
    """[Emax,...] slot array -> [128, Emax/128, ...] with slot s at [s%128, s//128]."""
    if a.ndim == 1:
        return np.ascontiguousarray(a.reshape(-1, 128).T)
    return np.ascontiguousarray(a.reshape(-1, 128, a.shape[-1]).transpose(1, 0, 2))


def _wrap_idx(a):
    """int16 idx list [n] -> [128, n/16] (16-partition wrap replicated 8x)."""
    w = a.reshape(-1, 16).T  # [16, n/16]
    return np.ascontiguousarray(np.tile(w, (8, 1)))




GMAX = 1024  # dma_gather num_idxs HW limit (crashes ~2048)


def _gather(nc, out3, in_ap, idx_tile, total_n, elem, elem_step=None):
    """Split a gather into <=GMAX-idx calls. out3: [128, total_n//128, elem]."""
    for j0 in range(0, total_n, GMAX):
        n = min(GMAX, total_n - j0)
        kw = {}
        if elem_step is not None:
            kw["elem_step"] = elem_step
        nc.gpsimd.dma_gather(
            out_ap=out3[:, j0 // 128 : (j0 + n) // 128, :],
            in_ap=in_ap,
            idxs_ap=idx_tile[:, j0 // 16 : (j0 + n) // 16],
            num_idxs=n, num_idxs_reg=n, elem_size=elem, **kw)


def _prep(edge_index, edge_attr):
    src = np.asarray(edge_index[0]).astype(np.int64)
    tgt = np.asarray(edge_index[1]).astype(np.int64)
    ea = np.asarray(edge_attr).astype(np.float32)

    core_of = src // NPC
    win_of = (src % NPC) // 128

    # per-(core,window,half) edge lists
    buckets = {}
    nlo_max = 0
    nhi_max = 0
    for k in range(NCORES):
        mk = core_of == k
        idx_k = np.nonzero(mk)[0]
        rot = (tgt[idx_k] - k * NPC) % NPAD
        hi = rot >= TSPLIT
        w = win_of[idx_k]
        for ww in range(NWIN):
            mw = w == ww
            elo = idx_k[mw & ~hi]
            ehi = idx_k[mw & hi]
            buckets[(k, ww)] = (elo, ehi)
            nlo_max = max(nlo_max, len(elo))
            nhi_max = max(nhi_max, len(ehi))

    Klo = max(1, -(-nlo_max // 128))
    Khi = max(1, -(-nhi_max // 128))
    KT = Klo + Khi
    EMAXW = KT * 128
    Emax = NWIN * EMAXW

    per_core = []
    for k in range(NCORES):
        tgt16 = np.zeros(Emax, np.int16)
        srcw = np.zeros(Emax, np.int16)
        mask = np.zeros(Emax, np.float32)
        ea8 = np.zeros((Emax, EA_DIM), np.float32)
        for ww in range(NWIN):
            elo, ehi = buckets[(k, ww)]
            base = ww * EMAXW
            for half, edges in ((0, elo), (1, ehi)):
                off = base + (0 if half == 0 else Klo * 128)
                n = len(edges)
                sl = slice(off, off + n)
                rot = (tgt[edges] - k * NPC) % NPAD
                tgt16[sl] = (rot if half == 0 else rot - HIB).astype(np.int16)
                srcw[sl] = (src[edges] - k * NPC - 128 * ww).astype(np.int16)
                mask[sl] = 1.0
                ea8[sl] = ea[edges]
        # gather idx blocks per window
        tlo = np.zeros((NWIN, 128, Klo * 8), np.int16)
        thi = np.zeros((NWIN, 128, Khi * 8), np.int16)
        tsw = np.zeros((NWIN, 128, KT * 8), np.int16)
        for ww in range(NWIN):
            b = ww * EMAXW
            tlo[ww] = _wrap_idx(tgt16[b : b + Klo * 128])
            thi[ww] = _wrap_idx(tgt16[b + Klo * 128 : b + KT * 128])
            tsw[ww] = _wrap_idx(srcw[b : b + KT * 128])
        rotoffs = ((128 * np.arange(NT, dtype=np.int64) - k * NPC) % NPAD).astype(
            np.int32
        )
        per_core.append(
            dict(
                tgtlo_idx=tlo,
                tgthi_idx=thi,
                srcw_idx=tsw,
                srcwin_f=_wrap_slots(srcw.astype(np.float32)),
                mask_w=_wrap_slots(mask),
                ea8T=np.ascontiguousarray(ea8.T),
                rotoffs=rotoffs.reshape(1, NT),
            )
        )
    return per_core, Klo, Khi


# ---------------------------------------------------------------- launch 1


def _build_launch1(Klo, Khi):
    KT = Klo + Khi
    Emax = NWIN * KT * 128
    NCHUNK = NWIN * KT

    nc = bacc.Bacc("TRN2", target_bir_lowering=False, debug=False, num_devices=NCORES)
    din = lambda n, s, d=F32: nc.dram_tensor(n, s, d, kind="ExternalInput")
    XT = din("XT", [NODE_DIM + 1, NPAD])            # row 16 = ones
    WN = din("WN", [NODE_DIM + 1, EMB])             # row 16 = b_node
    LNP = din("LNP", [128, 2 * EMB])                # g_node,beta_node replicated
    LNE = din("LNE", [128, 2 * 512])                # g_edge,beta_edge rep+tiled 8x
    WALL = din("WALL", [EMB, HEADS * OUT])
    WTALL = din("WTALL", [EMB, HEADS * OUT])        # block i = gat_W[i].T
    A2 = din("A2", [EMB, 2 * HEADS])                # col 2i/2i+1 = a_src/a_tgt head i
    AE9 = din("AE9", [EMB, 16])                     # col j = a_edge unit j (9 used)
    WE = din("WE", [EA_DIM + 1, EMB])               # row 8 = b_edge
    OW = din("OW", [HEADS * OUT, OUT])
    OWT = din("OWT", [OUT, HEADS * OUT])
    A2O = din("A2O", [EMB, 2])
    EA8T = din("EA8T", [EA_DIM + 1, Emax])          # row 8 = ones
    SRCWF = din("SRCWF", [128, Emax // 128])
    MASKW = din("MASKW", [128, Emax // 128])
    TLO = din("TLO", [NWIN, 128, Klo * 8], I16)
    THI = din("THI", [NWIN, 128, Khi * 8], I16)
    TSW = din("TSW", [NWIN, 128, KT * 8], I16)
    ROFF = din("ROFF", [1, NT], I32)

    H8 = nc.dram_tensor("H8", [NPAD, HEADS * OUT], BF16, kind="Internal")
    ATAB = nc.dram_tensor("ATAB", [NPAD, 64], F32, kind="Internal")
    HOUTS = nc.dram_tensor("HOUTS", [NPC, 128], F32, kind="ExternalOutput")
    ESC9 = nc.dram_tensor("ESC9", [128, NCHUNK], F32, kind="ExternalOutput")

    with tile.TileContext(nc) as tc:
        with tc.tile_pool(name="const", bufs=1) as cpool:
            ident = cpool.tile([128, 128], F32)
            make_identity(nc, ident[:])
            iota = cpool.tile([128, 128], F32)
            nc.gpsimd.iota(iota[:], pattern=[[1, 128]], base=0, channel_multiplier=0,
                           allow_small_or_imprecise_dtypes=True)
            negone = cpool.tile([128, 1], F32)
            nc.gpsimd.memset(negone[:], -1.0)
            wn_sb = cpool.tile([NODE_DIM + 1, EMB], F32)
            nc.sync.dma_start(wn_sb[:], WN[:])
            lnp_sb = cpool.tile([128, 2 * EMB], F32)
            nc.sync.dma_start(lnp_sb[:], LNP[:])
            lne_sb = cpool.tile([128, 2 * 512], F32)
            nc.sync.dma_start(lne_sb[:], LNE[:])
            wall_bf = cpool.tile([EMB, HEADS * OUT], BF16)
            nc.gpsimd.dma_start(wall_bf[:], WALL[:])  # SWDGE casts f32->bf16
            wtall_sb = cpool.tile([EMB, HEADS * OUT], F32)
            nc.sync.dma_start(wtall_sb[:], WTALL[:])
            a2_sb = cpool.tile([EMB, 2 * HEADS], F32)
            nc.sync.dma_start(a2_sb[:], A2[:])
            ae9_bf = cpool.tile([EMB, 16], BF16)
            nc.gpsimd.dma_start(ae9_bf[:], AE9[:])
            we_sb = cpool.tile([EA_DIM + 1, EMB], F32)
            nc.sync.dma_start(we_sb[:], WE[:])
            ow_sb = cpool.tile([128, 4, OUT], F32)
            nc.sync.dma_start(ow_sb[:], OW[:].rearrange("(a b) c -> b a c", b=128))
            owt_sb = cpool.tile([OUT, HEADS * OUT], F32)
            nc.sync.dma_start(owt_sb[:], OWT[:])
            a2o_sb = cpool.tile([EMB, 2], F32)
            nc.sync.dma_start(a2o_sb[:], A2O[:])
            roff_sb = cpool.tile([1, NT], I32)
            nc.sync.dma_start(roff_sb[:], ROFF[:])

            # Wa [64,16]: cols i / 8+i = W_i @ a_src_i / W_i @ a_tgt_i
            wa_sb = cpool.tile([EMB, 16], F32)
            owa_sb = cpool.tile([128, 4, 2], F32)  # OWa chunks: out_W@a2o
            with tc.tile_pool(name="wa_ps", bufs=2, space="PSUM") as wps:
                for i in range(HEADS):
                    p = wps.tile([EMB, 2], F32)
                    nc.tensor.matmul(p[:], lhsT=wtall_sb[:, 64 * i : 64 * (i + 1)],
                                     rhs=a2_sb[:, 2 * i : 2 * i + 2], start=True, stop=True)
                    nc.scalar.activation(wa_sb[:, i : i + 1], p[:, 0:1], AF.Copy)
                    nc.scalar.activation(wa_sb[:, 8 + i : 9 + i], p[:, 1:2], AF.Copy)
                for j in range(4):
                    p = wps.tile([128, 2], F32)
                    nc.tensor.matmul(p[:], lhsT=owt_sb[:, 128 * j : 128 * (j + 1)],
                                     rhs=a2o_sb[:], start=True, stop=True)
                    nc.scalar.activation(owa_sb[:, j, :], p[:], AF.Copy)

            # ---------------- stage A: x = MLP(X); H8, ATAB tables (rotated)
            K_NT = int(os.environ.get("K_NT", str(NT)))
            K_STATIC = os.environ.get("K_STATIC", "0") == "1"
            with (
                tc.tile_pool(name="sa", bufs=3) as sa,
                tc.tile_pool(name="sa_ps", bufs=2, space="PSUM") as sap,
            ):
                K_BODY = int(os.environ.get("K_BODY", "5"))
                for t in range(K_NT):
                    xt = sa.tile([NODE_DIM + 1, 128], F32)
                    nc.sync.dma_start(xt[:], XT[:, 128 * t : 128 * (t + 1)])
                    x_ps = sap.tile([128, EMB], F32, tag="xps")
                    nc.tensor.matmul(x_ps[:], lhsT=xt[:], rhs=wn_sb[:], start=True, stop=True)
                    xc = sa.tile([128, EMB], F32, tag="xc")
                    if K_BODY >= 2:
                        mu = sa.tile([128, 1], F32, tag="mu")
                        nc.vector.tensor_reduce(mu[:], x_ps[:], axis=AX.X, op=OP.add)
                        nc.vector.tensor_scalar_mul(mu[:], mu[:], -1.0 / EMB)
                        nc.vector.tensor_scalar(xc[:], x_ps[:], mu[:], None, op0=OP.add)
                        ssq = sa.tile([128, 1], F32, tag="ssq")
                        sc = sa.tile([128, EMB], F32, tag="sc")
                        nc.vector.tensor_tensor(out=sc[:], in0=xc[:], in1=xc[:], op=OP.mult)
                        nc.vector.tensor_reduce(ssq[:], sc[:], axis=AX.X, op=OP.add)
                        nc.vector.tensor_scalar(ssq[:], ssq[:], 1.0 / EMB, LN_EPS,
                                                op0=OP.mult, op1=OP.add)
                        nc.scalar.activation(ssq[:], ssq[:], AF.Sqrt)
                        nc.vector.reciprocal(ssq[:], ssq[:])
                        xn = sa.tile([128, EMB], F32, tag="xn")
                        nc.scalar.activation(xn[:], xc[:], AF.Copy, scale=ssq[:])
                        nc.vector.tensor_tensor(out=xn[:], in0=xn[:], in1=lnp_sb[:, :EMB], op=OP.mult)
                        nc.vector.tensor_tensor(out=xn[:], in0=xn[:], in1=lnp_sb[:, EMB:], op=OP.add)
                        xf = sa.tile([128, EMB], F32, tag="xf")
                        nc.scalar.activation(xf[:], xn[:], AF.Relu)
                    else:
                        xf = sa.tile([128, EMB], F32, tag="xf")
                        nc.scalar.activation(xf[:], x_ps[:], AF.Copy)
                    # transpose x tile
                    xT_f = sa.tile([EMB, 128], F32, tag="xTf")
                    xT_bf = sa.tile([EMB, 128], BF16, tag="xTbf")
                    if K_BODY >= 3:
                        xT_ps = sap.tile([128, 128], F32, tag="xTps")
                        nc.tensor.transpose(out=xT_ps[:EMB, :], in_=xf[:], identity=ident[:])
                        nc.scalar.activation(xT_f[:], xT_ps[:EMB, :], AF.Copy)
                        nc.vector.tensor_copy(xT_bf[:], xT_ps[:EMB, :])
                    else:
                        nc.vector.memset(xT_f[:], 0.0)
                        nc.vector.memset(xT_bf[:], 0.0)
                    # h for 8 heads
                    h_bf = sa.tile([128, HEADS * OUT], BF16, tag="hbf")
                    if K_BODY >= 4:
                        h_ps = sap.tile([128, HEADS * OUT], F32, tag="hps")
                        nc.tensor.matmul(h_ps[:], lhsT=xT_bf[:], rhs=wall_bf[:], start=True, stop=True)
                        nc.scalar.activation(h_bf[:], h_ps[:], AF.Copy)
                    else:
                        nc.vector.memset(h_bf[:], 0.0)
                    # alphas
                    a_sb = sa.tile([128, 64], F32, tag="asb")
                    nc.vector.memset(a_sb[:, 16:64], 0.0)
                    if K_BODY >= 5:
                        a_ps = sap.tile([128, 16], F32, tag="aps")
                        nc.tensor.matmul(a_ps[:], lhsT=xT_f[:], rhs=wa_sb[:], start=True, stop=True)
                        nc.scalar.activation(a_sb[:, 0:16], a_ps[:], AF.Copy)
                    else:
                        nc.vector.memset(a_sb[:, 0:16], 0.0)
                    # rotated writes
                    if K_STATIC:
                        nc.sync.dma_start(out=H8[128 * t : 128 * (t + 1), :], in_=h_bf[:])
                        nc.sync.dma_start(out=ATAB[128 * t : 128 * (t + 1), :], in_=a_sb[:])
                    else:
                        rv = nc.sync.alloc_register(f"roff{t}")
                        nc.sync.reg_load(rv, roff_sb[0:1, t : t + 1])
                        off = nc.sync.snap(rv, donate=True, min_val=0, max_val=NPAD - 128)
                        nc.sync.dma_start(out=H8[bass.ds(off, 128), :], in_=h_bf[:])
                        nc.sync.dma_start(out=ATAB[bass.ds(off, 128), :], in_=a_sb[:])

            # ---------------- stage B: edge MLP -> esc9 (resident SBUF)
            PHASE = os.environ.get("K_PHASE", "full")
            NW_RUN = int(os.environ.get("K_NWIN", str(NWIN)))
            esc9 = cpool.tile([128, NCHUNK, 9], F32)
            if PHASE == "A" or os.environ.get("K_NOB") == "1":
                nc.gpsimd.memset(esc9[:], 0.0)
            with (
                tc.tile_pool(name="sb", bufs=3) as sb,
                tc.tile_pool(name="sb_ps", bufs=2, space="PSUM") as sbp,
                tc.tile_pool(name="sbt_ps", bufs=3, space="PSUM") as sbtp,
            ):
                nmega = 0 if (PHASE == "A" or os.environ.get("K_NOB") == "1") else (NCHUNK + 7) // 8
                for m in range(nmega):
                    c0 = 8 * m
                    nch = min(8, NCHUNK - c0)
                    et = sb.tile([EA_DIM + 1, 8 * 128], F32, tag="et")
                    nc.sync.dma_start(et[:, : nch * 128],
                                      EA8T[:, c0 * 128 : (c0 + nch) * 128])
                    y_ps = sbp.tile([128, 512], F32, tag="yps")
                    for c in range(nch):
                        nc.tensor.matmul(y_ps[:, 64 * c : 64 * (c + 1)],
                                         lhsT=et[:, 128 * c : 128 * (c + 1)],
                                         rhs=we_sb[:], start=True, stop=True)
                    w = nch * 64
                    y3 = y_ps[:].rearrange("p (c f) -> p c f", f=64)[:, :nch, :]
                    mu = sb.tile([128, 8], F32, tag="mu")
                    nc.vector.tensor_reduce(mu[:, :nch], y3, axis=AX.X, op=OP.add)
                    nc.vector.tensor_scalar_mul(mu[:, :nch], mu[:, :nch], -1.0 / EMB)
                    xc = sb.tile([128, 512], F32, tag="xc")
                    xc3 = xc[:].rearrange("p (c f) -> p c f", f=64)[:, :nch, :]
                    nc.vector.tensor_tensor(out=xc3, in0=y3,
                                            in1=mu[:, :nch].to_broadcast([128, nch, 64]),
                                            op=OP.add)
                    ssq = sb.tile([128, 8], F32, tag="ssq")
                    sc = sb.tile([128, 512], F32, tag="sc")
                    sc3 = sc[:].rearrange("p (c f) -> p c f", f=64)[:, :nch, :]
                    nc.vector.tensor_tensor(out=sc3, in0=xc3, in1=xc3, op=OP.mult)
                    nc.vector.tensor_reduce(ssq[:, :nch], sc3, axis=AX.X, op=OP.add)
                    nc.vector.tensor_scalar(ssq[:, :nch], ssq[:, :nch], 1.0 / EMB,
                                            LN_EPS, op0=OP.mult, op1=OP.add)
                    nc.scalar.activation(ssq[:, :nch], ssq[:, :nch], AF.Sqrt)
                    nc.vector.reciprocal(ssq[:, :nch], ssq[:, :nch])
                    nc.vector.tensor_tensor(out=xc3, in0=xc3,
                                            in1=ssq[:, :nch].to_broadcast([128, nch, 64]),
                                            op=OP.mult)
                    nc.vector.tensor_tensor(out=xc[:, :w], in0=xc[:, :w],
                                            in1=lne_sb[:, :w], op=OP.mult)
                    nc.vector.tensor_tensor(out=xc[:, :w], in0=xc[:, :w],
                                            in1=lne_sb[:, 512 : 512 + w], op=OP.add)
                    zf = sb.tile([128, 512], F32, tag="zf")
                    nc.scalar.activation(zf[:, :w], xc[:, :w], AF.Relu)
                    for c in range(nch):
                        zT_ps = sbtp.tile([EMB, 128], F32, tag="zTps")
                        nc.tensor.transpose(out=zT_ps[:], in_=zf[:, 64 * c : 64 * (c + 1)],
                                            identity=ident[:])
                        zT = sb.tile([EMB, 128], BF16, tag="zT")
                        nc.vector.tensor_copy(zT[:], zT_ps[:])
                        e_ps = sbtp.tile([128, 16], F32, tag="eps")
                        nc.tensor.matmul(e_ps[:], lhsT=zT[:], rhs=ae9_bf[:],
                                         start=True, stop=True)
                        nc.scalar.activation(esc9[:, c0 + c, :], e_ps[:, 0:9], AF.Copy)
            # col 8 of esc9 -> DRAM for launch 2 (store as f32)
            with tc.tile_pool(name="e9", bufs=1) as e9p:
                e9 = e9p.tile([128, NCHUNK], F32)
                nc.vector.tensor_copy(e9[:], esc9[:, :, 8])
                nc.sync.dma_start(ESC9[:, :], e9[:])

            # ---------------- stage C: per-window edge pass (8 heads)
            if PHASE != "full" or NW_RUN < NWIN:
                zz = cpool.tile([128, 128], F32)
                nc.vector.memset(zz[:], 0.0)
                for w in range(NWIN if PHASE != "full" else NW_RUN, NWIN):
                    pass
                for w in range(0 if PHASE != "full" else NW_RUN, NWIN):
                    nc.sync.dma_start(HOUTS[128 * w : 128 * (w + 1), :], zz[:])
            srcwf_sb = cpool.tile([128, Emax // 128], F32)
            nc.sync.dma_start(srcwf_sb[:], SRCWF[:])
            maskw_sb = cpool.tile([128, Emax // 128], F32)
            nc.sync.dma_start(maskw_sb[:], MASKW[:])
            with (
                tc.tile_pool(name="ec", bufs=2) as ec,
                tc.tile_pool(name="ecs", bufs=3) as ecs,
                tc.tile_pool(name="ec_ps", bufs=2, space="PSUM") as ecp,
                tc.tile_pool(name="ed_ps", bufs=1, space="PSUM") as edp,
                tc.tile_pool(name="et_ps", bufs=2, space="PSUM") as etp,
            ):
                K_EDGE = int(os.environ.get("K_EDGE", "4"))
                for w in range(NW_RUN if PHASE == "full" else 0):
                    cw0 = w * KT
                    ilo = ec.tile([128, Klo * 8], I16, tag="ilo")
                    nc.sync.dma_start(ilo[:], TLO[w])
                    ihi = ec.tile([128, Khi * 8], I16, tag="ihi")
                    nc.sync.dma_start(ihi[:], THI[w])
                    isw = ec.tile([128, KT * 8], I16, tag="isw")
                    nc.sync.dma_start(isw[:], TSW[w])
                    g_src = ec.tile([128, KT, 64], F32, tag="gsrc")
                    _gather(nc, g_src[:], ATAB[128 * w : 128 * (w + 1), :], isw, KT * 128, 64)
                    g_tlo = ec.tile([128, Klo, 64], F32, tag="gtlo")
                    _gather(nc, g_tlo[:], ATAB[0:TSPLIT, :], ilo, Klo * 128, 64)
                    g_thi = ec.tile([128, Khi, 64], F32, tag="gthi")
                    _gather(nc, g_thi[:], ATAB[HIB:NPAD, :], ihi, Khi * 128, 64)
                    G_lo = ec.tile([128, Klo, 512], BF16, tag="Glo")
                    _gather(nc, G_lo[:], H8[0:TSPLIT, :], ilo, Klo * 128, 512)
                    G_hi = ec.tile([128, Khi, 512], BF16, tag="Ghi")
                    _gather(nc, G_hi[:], H8[HIB:NPAD, :], ihi, Khi * 128, 512)
                    if K_EDGE == 1:
                        hr0 = ecs.tile([128, 128], F32, tag="hrow")
                        nc.vector.tensor_copy(hr0[:, 0:64], g_src[:, 0, :])
                        nc.vector.tensor_copy(hr0[:, 64:128], g_tlo[:, 0, :])
                        nc.vector.tensor_tensor(out=hr0[:, 0:64], in0=hr0[:, 0:64],
                                                in1=g_thi[:, 0, :], op=OP.add)
                        nc.vector.tensor_tensor(out=hr0[:, 0:64], in0=hr0[:, 0:64],
                                                in1=G_lo[:, 0, 0:64], op=OP.add)
                        nc.vector.tensor_tensor(out=hr0[:, 0:64], in0=hr0[:, 0:64],
                                                in1=G_hi[:, 0, 0:64], op=OP.add)
                        nc.sync.dma_start(HOUTS[128 * w : 128 * (w + 1), :], hr0[:])
                        continue
                    # scores
                    s8 = ecs.tile([128, KT, 8], F32, tag="s8")
                    nc.vector.tensor_tensor(out=s8[:, :Klo, :], in0=g_src[:, :Klo, 0:8],
                                            in1=g_tlo[:, :, 8:16], op=OP.add)
                    nc.vector.tensor_tensor(out=s8[:, Klo:, :], in0=g_src[:, Klo:, 0:8],
                                            in1=g_thi[:, :, 8:16], op=OP.add)
                    nc.vector.tensor_tensor(out=s8[:], in0=s8[:],
                                            in1=esc9[:, cw0 : cw0 + KT, 0:8], op=OP.add)
                    lr = ecs.tile([128, KT, 8], F32, tag="lr")
                    nc.vector.tensor_scalar_mul(lr[:], s8[:], SLOPE)
                    nc.vector.tensor_tensor(out=s8[:], in0=s8[:], in1=lr[:], op=OP.max)
                    nc.scalar.activation(s8[:], s8[:], AF.Exp)
                    nc.vector.tensor_tensor(
                        out=s8[:], in0=s8[:],
                        in1=maskw_sb[:, cw0 : cw0 + KT].to_broadcast([128, KT, 8]),
                        op=OP.mult)
                    w8 = ecs.tile([128, KT, 8], BF16, tag="w8")
                    nc.vector.tensor_copy(w8[:], s8[:])
                    if K_EDGE == 2:
                        hr0 = ecs.tile([128, 128], F32, tag="hrow")
                        nc.vector.memset(hr0[:], 0.0)
                        nc.vector.tensor_copy(hr0[:, 0:22], s8[:, :, 0])
                        nc.sync.dma_start(HOUTS[128 * w : 128 * (w + 1), :], hr0[:])
                        continue
                    num_ps = ecp.tile([128, 512], F32, tag="num")
                    den_ps = edp.tile([128, 8], F32, tag="den")
                    for c in range(KT):
                        S = ecs.tile([128, 128], BF16, tag="S")
                        nc.vector.tensor_scalar(
                            S[:], iota[:], srcwf_sb[:, cw0 + c : cw0 + c + 1], None,
                            op0=OP.is_equal)
                        G = G_lo[:, c, :] if c < Klo else G_hi[:, c - Klo, :]
                        V = ecs.tile([128, 512], BF16, tag="V")
                        nc.vector.tensor_tensor(
                            out=V[:].rearrange("p (i f) -> p i f", f=64),
                            in0=G.rearrange("p (i f) -> p i f", f=64),
                            in1=w8[:, c, :].to_broadcast([128, 8, 64]), op=OP.mult)
                        nc.tensor.matmul(num_ps[:], lhsT=S[:], rhs=V[:],
                                         start=(c == 0), stop=(c == KT - 1))
                        nc.tensor.matmul(den_ps[:], lhsT=S[:], rhs=w8[:, c, :],
                                         start=(c == 0), stop=(c == KT - 1))
                    if K_EDGE == 3:
                        hr0 = ecs.tile([128, 128], F32, tag="hrow")
                        nc.vector.tensor_copy(hr0[:, 0:64], num_ps[:, 0:64])
                        nc.vector.tensor_copy(hr0[:, 64:72], den_ps[:])
                        nc.vector.memset(hr0[:, 72:128], 0.0)
                        nc.sync.dma_start(HOUTS[128 * w : 128 * (w + 1), :], hr0[:])
                        continue
                    # xh = elu(elu(num/den))
                    den = ecs.tile([128, 8], F32, tag="dens")
                    nc.vector.tensor_scalar(den[:], den_ps[:], 1e-16, None, op0=OP.add)
                    nc.vector.reciprocal(den[:], den[:])
                    xh = ecs.tile([128, 512], F32, tag="xh")
                    nc.vector.tensor_tensor(
                        out=xh[:].rearrange("p (i f) -> p i f", f=64),
                        in0=num_ps[:].rearrange("p (i f) -> p i f", f=64),
                        in1=den[:].to_broadcast([128, 8, 64]), op=OP.mult)
                    m0 = ecs.tile([128, 512], F32, tag="m0")
                    nc.vector.tensor_scalar_min(m0[:], xh[:], 0.0)
                    nc.scalar.activation(m0[:], m0[:], AF.Exp)
                    nc.scalar.activation(m0[:], m0[:], AF.Exp, bias=negone[:])
                    r0 = ecs.tile([128, 512], F32, tag="r0")
                    nc.scalar.activation(r0[:], xh[:], AF.Relu)
                    nc.vector.tensor_scalar(m0[:], m0[:], -1.0, None, op0=OP.add)
                    nc.vector.tensor_tensor(out=xh[:], in0=m0[:], in1=r0[:], op=OP.add)
                    # h_out slice + out-layer alphas
                    ho_ps = edp.tile([128, OUT], F32, tag="ho")
                    ao_ps = edp.tile([128, 2], F32, tag="ao")
                    for j in range(4):
                        xT_ps = etp.tile([128, 128], F32, tag="xTps2")
                        nc.tensor.transpose(out=xT_ps[:], in_=xh[:, 128 * j : 128 * (j + 1)],
                                            identity=ident[:])
                        xT = ecs.tile([128, 128], F32, tag="xT2")
                        nc.scalar.activation(xT[:], xT_ps[:], AF.Copy)
                        nc.tensor.matmul(ho_ps[:], lhsT=xT[:], rhs=ow_sb[:, j, :],
                                         start=(j == 0), stop=(j == 3))
                        nc.tensor.matmul(ao_ps[:], lhsT=xT[:], rhs=owa_sb[:, j, :],
                                         start=(j == 0), stop=(j == 3))
                    hrow = ecs.tile([128, 128], F32, tag="hrow")
                    nc.vector.memset(hrow[:, OUT + 2 :], 0.0)
                    nc.scalar.activation(hrow[:, 0:OUT], ho_ps[:], AF.Copy)
                    nc.scalar.activation(hrow[:, OUT : OUT + 2], ao_ps[:], AF.Copy)
                    nc.sync.dma_start(HOUTS[128 * w : 128 * (w + 1), :], hrow[:])
    nc.compile()
    return nc


# ---------------------------------------------------------------- launch 2


def _build_launch2(Klo, Khi):
    KT = Klo + Khi
    Emax = NWIN * KT * 128
    NCHUNK = NWIN * KT

    nc = bacc.Bacc("TRN2", target_bir_lowering=False, debug=False, num_devices=NCORES)
    din = lambda n, s, d=F32: nc.dram_tensor(n, s, d, kind="ExternalInput")
    HR = din("HR", [NPAD, 128])      # rotated [h_out(64) | a_src | a_tgt | pad]
    E9 = din("E9", [128, NCHUNK])
    SRCWF = din("SRCWF", [128, Emax // 128])
    MASKW = din("MASKW", [128, Emax // 128])
    TLO = din("TLO", [NWIN, 128, Klo * 8], I16)
    THI = din("THI", [NWIN, 128, Khi * 8], I16)
    TSW = din("TSW", [NWIN, 128, KT * 8], I16)
    OUTT = nc.dram_tensor("OUTT", [NPC, OUT], F32, kind="ExternalOutput")

    with tile.TileContext(nc) as tc:
        with tc.tile_pool(name="const", bufs=1) as cpool:
            iota = cpool.tile([128, 128], F32)
            nc.gpsimd.iota(iota[:], pattern=[[1, 128]], base=0, channel_multiplier=0,
                           allow_small_or_imprecise_dtypes=True)
            e9_sb = cpool.tile([128, NCHUNK], F32)
            nc.sync.dma_start(e9_sb[:], E9[:])
            srcwf_sb = cpool.tile([128, Emax // 128], F32)
            nc.sync.dma_start(srcwf_sb[:], SRCWF[:])
            maskw_sb = cpool.tile([128, Emax // 128], F32)
            nc.sync.dma_start(maskw_sb[:], MASKW[:])
            with (
                tc.tile_pool(name="ec", bufs=2) as ec,
                tc.tile_pool(name="ecs", bufs=3) as ecs,
                tc.tile_pool(name="ec_ps", bufs=2, space="PSUM") as ecp,
                tc.tile_pool(name="ed_ps", bufs=1, space="PSUM") as edp,
            ):

                for w in range(NWIN):
                    cw0 = w * KT
                    ilo = ec.tile([128, Klo * 8], I16, tag="ilo")
                    nc.sync.dma_start(ilo[:], TLO[w])
                    ihi = ec.tile([128, Khi * 8], I16, tag="ihi")
                    nc.sync.dma_start(ihi[:], THI[w])
                    isw = ec.tile([128, KT * 8], I16, tag="isw")
                    nc.sync.dma_start(isw[:], TSW[w])
                    g_src = ec.tile([128, KT, 64], F32, tag="gsrc")
                    _gather(nc, g_src[:], HR[128 * w : 128 * (w + 1), 64:128], isw,
                            KT * 128, 64, elem_step=128)
                    g_tlo = ec.tile([128, Klo, 128], F32, tag="gtlo")
                    _gather(nc, g_tlo[:], HR[0:TSPLIT, :], ilo, Klo * 128, 128)
                    g_thi = ec.tile([128, Khi, 128], F32, tag="gthi")
                    _gather(nc, g_thi[:], HR[HIB:NPAD, :], ihi, Khi * 128, 128)
                    s1 = ecs.tile([128, KT], F32, tag="s1")
                    nc.vector.tensor_tensor(out=s1[:, :Klo], in0=g_src[:, :Klo, 0],
                                            in1=g_tlo[:, :, 65], op=OP.add)
                    nc.vector.tensor_tensor(out=s1[:, Klo:], in0=g_src[:, Klo:, 0],
                                            in1=g_thi[:, :, 65], op=OP.add)
                    nc.vector.tensor_tensor(out=s1[:], in0=s1[:],
                                            in1=e9_sb[:, cw0 : cw0 + KT], op=OP.add)
                    lr = ecs.tile([128, KT], F32, tag="lr")
                    nc.vector.tensor_scalar_mul(lr[:], s1[:], SLOPE)
                    nc.vector.tensor_tensor(out=s1[:], in0=s1[:], in1=lr[:], op=OP.max)
                    nc.scalar.activation(s1[:], s1[:], AF.Exp)
                    nc.vector.tensor_tensor(out=s1[:], in0=s1[:],
                                            in1=maskw_sb[:, cw0 : cw0 + KT], op=OP.mult)
                    w1 = ecs.tile([128, KT], BF16, tag="w1")
                    nc.vector.tensor_copy(w1[:], s1[:])
                    num_ps = ecp.tile([128, OUT], F32, tag="num")
                    den_ps = edp.tile([128, 1], F32, tag="den")
                    for c in range(KT):
                        S = ecs.tile([128, 128], BF16, tag="S")
                        nc.vector.tensor_scalar(
                            S[:], iota[:], srcwf_sb[:, cw0 + c : cw0 + c + 1], None,
                            op0=OP.is_equal)
                        G = g_tlo[:, c, 0:64] if c < Klo else g_thi[:, c - Klo, 0:64]
                        V = ecs.tile([128, OUT], BF16, tag="V")
                        nc.vector.tensor_tensor(out=V[:], in0=G,
                                                in1=w1[:, c : c + 1].to_broadcast([128, 64]),
                                                op=OP.mult)
                        nc.tensor.matmul(num_ps[:], lhsT=S[:], rhs=V[:],
                                         start=(c == 0), stop=(c == KT - 1))
                        nc.tensor.matmul(den_ps[:], lhsT=S[:], rhs=w1[:, c : c + 1],
                                         start=(c == 0), stop=(c == KT - 1))
                    den = ecs.tile([128, 1], F32, tag="dens")
                    nc.vector.tensor_scalar(den[:], den_ps[:], 1e-16, None, op0=OP.add)
                    nc.vector.reciprocal(den[:], den[:])
                    h2 = ecs.tile([128, OUT], F32, tag="h2")
                    nc.vector.tensor_tensor(out=h2[:], in0=num_ps[:],
                                            in1=den[:].to_broadcast([128, OUT]), op=OP.mult)
                    # elu
                    m0 = ecs.tile([128, OUT], F32, tag="m0")
                    nc.vector.tensor_scalar_min(m0[:], h2[:], 0.0)
                    nc.scalar.activation(m0[:], m0[:], AF.Exp)
                    r0 = ecs.tile([128, OUT], F32, tag="r0")
                    nc.scalar.activation(r0[:], h2[:], AF.Relu)
                    nc.vector.tensor_scalar(m0[:], m0[:], -1.0, None, op0=OP.add)
                    nc.vector.tensor_tensor(out=h2[:], in0=m0[:], in1=r0[:], op=OP.add)
                    # log_softmax over 64
                    negm = ecs.tile([128, 1], F32, tag="negm")
                    nc.vector.tensor_reduce(negm[:], h2[:], axis=AX.X, op=OP.max)
                    nc.vector.tensor_scalar_mul(negm[:], negm[:], -1.0)
                    etile = ecs.tile([128, OUT], F32, tag="etile")
                    ssum = ecs.tile([128, 1], F32, tag="ssum")
                    nc.scalar.activation(etile[:], h2[:], AF.Exp, bias=negm[:],
                                         accum_out=ssum[:])
                    nc.scalar.activation(ssum[:], ssum[:], AF.Ln)
                    d = ecs.tile([128, 1], F32, tag="d")
                    nc.vector.tensor_tensor(out=d[:], in0=negm[:], in1=ssum[:],
                                            op=OP.subtract)
                    res = ecs.tile([128, OUT], F32, tag="res")
                    nc.vector.tensor_scalar(res[:], h2[:], d[:], None, op0=OP.add)
                    nc.sync.dma_start(OUTT[128 * w : 128 * (w + 1), :], res[:])
    nc.compile()
    return nc


# ---------------------------------------------------------------- driver


def _make_inputs1(X, edge_attr, w_node, b_node, g_node, beta_node,
                  w_edge, b_edge, g_edge, beta_edge,
                  gat_W, gat_a, out_W, out_a, edge_index):
    X = np.asarray(X, np.float32)
    per_core, Klo, Khi = _prep(edge_index, edge_attr)

    # ---- shared (core-independent) inputs, host layout only
    Xp = np.zeros((NPAD, NODE_DIM + 1), np.float32)
    Xp[:N, :NODE_DIM] = X
    Xp[:, NODE_DIM] = 1.0
    XT = np.ascontiguousarray(Xp.T)
    WN = np.concatenate([np.asarray(w_node, np.float32),
                         np.asarray(b_node, np.float32)[None, :]], 0)
    WE = np.concatenate([np.asarray(w_edge, np.float32),
                         np.asarray(b_edge, np.float32)[None, :]], 0)
    LNP = np.concatenate([np.tile(np.asarray(g_node, np.float32), (128, 1)),
                          np.tile(np.asarray(beta_node, np.float32), (128, 1))], 1)
    LNE = np.concatenate([np.tile(np.asarray(g_edge, np.float32), (128, 8)),
                          np.tile(np.asarray(beta_edge, np.float32), (128, 8))], 1)
    gW = np.asarray(gat_W, np.float32)
    ga = np.asarray(gat_a, np.float32)
    oW = np.asarray(out_W, np.float32)
    oa = np.asarray(out_a, np.float32)
    WALL = np.concatenate([gW[i] for i in range(HEADS)], 1)
    WTALL = np.concatenate([gW[i].T for i in range(HEADS)], 1)
    A2 = np.zeros((EMB, 2 * HEADS), np.float32)
    for i in range(HEADS):
        A2[:, 2 * i] = ga[i, :OUT]
        A2[:, 2 * i + 1] = ga[i, OUT : 2 * OUT]
    AE9 = np.zeros((EMB, 16), np.float32)
    for i in range(HEADS):
        AE9[:, i] = ga[i, 2 * OUT :]
    AE9[:, 8] = oa[2 * OUT :]
    A2O = np.stack([oa[:OUT], oa[OUT : 2 * OUT]], 1)
    OWT = np.ascontiguousarray(oW.T)

    shared = dict(XT=XT, WN=WN, LNP=LNP, LNE=LNE, WALL=WALL, WTALL=WTALL,
                  A2=A2, AE9=AE9, WE=WE, OW=oW, OWT=OWT, A2O=A2O)

    in_maps = []
    for k in range(NCORES):
        pc = per_core[k]
        in_maps.append({**{kk: np.ascontiguousarray(vv) for kk, vv in shared.items()},
                        "EA8T": np.concatenate(
                            [pc["ea8T"], np.ones((1, pc["ea8T"].shape[1]), np.float32)], 0),
                        "SRCWF": pc["srcwin_f"], "MASKW": pc["mask_w"],
                        "TLO": pc["tgtlo_idx"], "THI": pc["tgthi_idx"],
                        "TSW": pc["srcw_idx"], "ROFF": pc["rotoffs"]})
    return in_maps, per_core, Klo, Khi


def kernel(X, edge_attr, w_node, b_node, g_node, beta_node,
           w_edge, b_edge, g_edge, beta_edge,
           gat_W, gat_a, out_W, out_a,
           edge_index, matched_car_infra_nodes):
    in_maps, per_core, Klo, Khi = _make_inputs1(
        X, edge_attr, w_node, b_node, g_node, beta_node,
        w_edge, b_edge, g_edge, beta_edge,
        gat_W, gat_a, out_W, out_a, edge_index)
    import time as _time
    nc1 = _build_launch1(Klo, Khi)
    kernel.nc1 = nc1
    _t = _time.perf_counter()
    res1 = run_bass_kernel_spmd(nc1, in_maps, core_ids=list(range(NCORES)))
    kernel.wall1 = _time.perf_counter() - _t

    # host: assemble global HOUT and rotate per core
    HG = np.zeros((NPAD, 128), np.float32)
    for k in range(NCORES):
        lo = k * NPC
        hi = min((k + 1) * NPC, NPAD)
        HG[lo:hi] = res1.results[k]["HOUTS"][: hi - lo]

    nc2 = _build_launch2(Klo, Khi)
    in_maps2 = []
    for k in range(NCORES):
        pc = per_core[k]
        HR = np.ascontiguousarray(np.roll(HG, -k * NPC, axis=0))
        in_maps2.append({"HR": HR, "E9": res1.results[k]["ESC9"],
                         "SRCWF": pc["srcwin_f"], "MASKW": pc["mask_w"],
                         "TLO": pc["tgtlo_idx"], "THI": pc["tgthi_idx"],
                         "TSW": pc["srcw_idx"]})
    kernel.nc2 = nc2
    _t = _time.perf_counter()
    res2 = run_bass_kernel_spmd(nc2, in_maps2, core_ids=list(range(NCORES)))
    kernel.wall2 = _time.perf_counter() - _t

    out = np.zeros((N, OUT), np.float32)
    for k in range(NCORES):
        lo = k * NPC
        hi = min((k + 1) * NPC, N)
        out[lo:hi] = res2.results[k]["OUTT"][: hi - lo]
    return out

